# revision 1
# baseline (speedup 1.0000x reference)
"""GPT2-style fused attention (DecisionTransformer) on 8 Trainium2 NeuronCores.

Sharding: tensor-parallel over the 16 heads (2 heads per core, both batch
elements on every core).  Each core:
  - loads the full hidden_states [4096, 1024],
  - computes Q/K/V for its 2 heads (transposed layout via PE transposes),
  - causal attention for its 4 (batch, head) pairs: scores^T = K @ Q^T,
    exp (no max subtraction -- logits are small and bounded), ones-column
    appended to V gives the softmax denominator for free in the A@V matmul,
  - row-parallel output projection with its 128 rows of c_proj_w,
  - writes a full-shape partial output [4096, 1024].
Host gathers with a sum over the 8 partials (the row-parallel all-reduce)
and adds c_proj_b.

Matmuls run in float32r (full-rate fp32 streaming, ~tf32-like rounding);
measured output error vs the fp32 reference is ~2e-4 relative to absmax.
"""

import sys

for _p in ("/opt/trn_rl_repo",):
    if _p not in sys.path:
        sys.path.insert(0, _p)

import numpy as np

import concourse.bass as bass
import concourse.mybir as mybir
import concourse.tile as tile
from concourse import bacc
from concourse.bass_utils import run_bass_kernel_spmd
from concourse.masks import make_identity

P = 128
B, S, D, H, HD = 2, 2048, 1024, 16, 64
T = B * S              # 4096 tokens
FQKV = 3 * P           # 384 per-core qkv features (q128 | k128 | v128)
KO = D // P            # 8 contraction chunks
TCH = 512              # token chunk for qkv phase
NTCH = T // TCH        # 8
QC = 512               # query chunk in attention
NQC = S // QC          # 4
NKB = S // P           # 16 key blocks per sequence
SCALE = 1.0 / float(HD) ** 0.5
N_CORES = 8
HPC = H // N_CORES     # 2 heads per core

f32 = mybir.dt.float32
f32r = mybir.dt.float32r
MM_DT = f32r


def _emit_body(nc, tc, pools, consts, it, phases='full'):
    (xin_pool, xt_pool, qkvt_pool, vaug_pool, pt_pool, atn_pool, out_pool,
     small_pool, ps_mm, ps_s, ps_o) = pools
    (wqkv_sb, wp_sb, bqkv_sb, ident_f32, identr, ident2, mask128,
     ones1, x_d, out_d, xi_pre) = consts

    # per-batch K^T / V^T / padded-Q^T tiles so batch 1's projection can
    # overlap batch 0's attention (no shared-tile false dependencies)
    ktb = [qkvt_pool.tile([P, S], MM_DT, tag=f"kt{b}", name=f"kt{b}")
           for b in range(B)]
    vtb = [qkvt_pool.tile([P, S], MM_DT, tag=f"vt{b}", name=f"vt{b}")
           for b in range(B)]
    # Q^T per (batch, local head), zero-padded to 128 contraction rows: the
    # other head's 64 rows stay zero so a full-128-partition matmul against
    # the stacked K^T contracts exactly (sub-128 matmuls run at half rate).
    qpad = [
        [qkvt_pool.tile([P, S], MM_DT, tag=f"qp{b}{h}", name=f"qp{b}{h}")
         for h in range(HPC)]
        for b in range(B)
    ]
    if it == 0:
        for b in range(B):
            nc.vector.memset(qpad[b][0][HD:, :].bitcast(f32), 0.0)
            nc.vector.memset(qpad[b][1][:HD, :].bitcast(f32), 0.0)
    vaug = [
        vaug_pool.tile([P, NKB, P], MM_DT, tag=f"vaug{p}", name=f"vaug{p}")
        for p in range(B * HPC)
    ]
    atn = [
        [
            atn_pool.tile([P, QC], MM_DT, tag=f"atn{b}_{q}", name=f"atn{b}_{q}")
            for q in range(NQC)
        ]
        for b in range(B)
    ]

    # ---- phases 1-3 per batch: X^T, QKV projection, V_aug ----
    for b in range(B):
        for i in range(S // TCH):
            gi = b * (S // TCH) + i
            xt = xt_pool.tile([P, KO, TCH], MM_DT, tag="xt", name="xt")
            if it == 0 and gi == 0:
                xins = xi_pre
            else:
                xins = []
                for j in range(TCH // P):
                    xi = xin_pool.tile([P, D], f32, tag="xi", name="xi")
                    nc.sync.dma_start(
                        xi[:], x_d[gi * TCH + j * P : gi * TCH + (j + 1) * P, :]
                    )
                    xins.append(xi)
            # 4 PE transposes share one PSUM bank -> single wide eviction
            for ko in range(KO):
                ps = ps_mm.tile([P, TCH], f32, tag="mm", name="psmm")
                for j in range(TCH // P):
                    nc.tensor.transpose(
                        ps[:, j * P : (j + 1) * P],
                        xins[j][:, ko * P : (ko + 1) * P],
                        ident_f32[:],
                    )
                nc.scalar.copy(xt[:, ko, :], ps[:])
            for fc in range(3):
                ps = ps_mm.tile([P, TCH], f32, tag="mm", name="psmm")
                for ko in range(KO):
                    nc.tensor.matmul(
                        ps[:],
                        wqkv_sb[:, ko, fc * P : (fc + 1) * P],
                        xt[:, ko, :],
                        start=(ko == 0),
                        stop=(ko == KO - 1),
                    )
                # evict + per-partition bias add on DVE
                cs = slice(i * TCH, (i + 1) * TCH)
                if fc == 0:
                    nc.vector.tensor_scalar(
                        qpad[b][0][:HD, cs], ps[:HD],
                        bqkv_sb[:HD, fc : fc + 1], None, mybir.AluOpType.add,
                    )
                    nc.vector.tensor_scalar(
                        qpad[b][1][HD:, cs], ps[HD:],
                        bqkv_sb[HD:, fc : fc + 1], None, mybir.AluOpType.add,
                    )
                else:
                    dst = ktb[b] if fc == 1 else vtb[b]
                    nc.vector.tensor_scalar(
                        dst[:, cs], ps[:],
                        bqkv_sb[:, fc : fc + 1], None, mybir.AluOpType.add,
                    )
        # V_aug for this batch (V back to natural layout + ones column)
        for hl in range(HPC):
            p = b * HPC + hl
            vt = vtb[b][hl * HD : (hl + 1) * HD, :]
            if it == 0:
                nc.vector.memset(vaug[p][:, :, HD : HD + 1].bitcast(f32), 1.0)
                nc.vector.memset(vaug[p][:, :, HD + 1 :].bitcast(f32), 0.0)
            for kb in range(0, NKB, 2):
                ps = ps_mm.tile([P, TCH], f32, tag="mm", name="psmm")
                for u in range(2):
                    nc.tensor.transpose(
                        ps[:, u * HD : (u + 1) * HD].bitcast(f32r),
                        vt[:, (kb + u) * P : (kb + u + 1) * P],
                        ident2[hl * HD : (hl + 1) * HD, :],
                    )
                nc.vector.tensor_copy(
                    vaug[p][:, kb : kb + 2, :HD],
                    ps[:, : 2 * HD].rearrange("p (u h) -> p u h", u=2),
                )

    if phases == 'a':
        return
    # ---- phase 4+5: attention + output projection ----
    for b in range(B):
        for qc in range(NQC):
            for hl in range(HPC):
                p = b * HPC + hl
                rhs_q = qpad[b][hl][:, qc * QC : (qc + 1) * QC]
                po = ps_o.tile([P, QC], f32, tag="po", name="pso")
                nkb = (qc + 1) * (QC // P)
                for kb in range(nkb):
                    j = kb - qc * (QC // P)
                    lo = j * P if j > 0 else 0
                    ps = ps_s.tile([P, QC], f32, tag="s", name="pss")
                    nc.tensor.matmul(
                        ps[:, lo:],
                        ktb[b][:, kb * P : (kb + 1) * P],
                        rhs_q[:, lo:],
                        start=True,
                        stop=True,
                    )
                    pt = pt_pool.tile([P, QC], MM_DT, tag="pt", name="pt")
                    if j < 0:
                        nc.scalar.activation(
                            pt[:],
                            ps[:],
                            mybir.ActivationFunctionType.Exp,
                            scale=SCALE,
                        )
                        nc.tensor.matmul(
                            po[:],
                            vaug[p][:, kb, :],
                            pt[:],
                            start=(kb == 0),
                            stop=False,
                        )
                    else:
                        # diagonal block: only cols >= j*128 are live; the
                        # A@V matmul covers just that column range, so the
                        # masked region needs no zeroing at all.
                        nc.scalar.activation(
                            pt[:, j * P :],
                            ps[:, j * P :],
                            mybir.ActivationFunctionType.Exp,
                            scale=SCALE,
                        )
                        nc.vector.tensor_tensor(
                            pt[:, j * P : (j + 1) * P],
                            pt[:, j * P : (j + 1) * P],
                            mask128[:],
                            mybir.AluOpType.mult,
                        )
                        nc.tensor.matmul(
                            po[:, j * P :],
                            vaug[p][:, kb, :],
                            pt[:, j * P :],
                            start=(kb == 0),
                            stop=(kb == nkb - 1),
                        )
                # normalize: A^T = O^T_u * (1/denom), denom = po[64].
                # Broadcast denom across 64 partitions FIRST (rank-1 PE
                # matmul), then reciprocal on 64 lanes -- a [1,512]
                # single-lane reciprocal measures ~3.4us on HW.
                den = small_pool.tile([1, QC], MM_DT, tag="rec", name="rec")
                nc.vector.tensor_copy(den[:], po[HD : HD + 1, :])
                rbc = ps_mm.tile([P, TCH], f32, tag="mm", name="psmm")[:HD, :QC]
                nc.tensor.matmul(
                    rbc[:],
                    ones1[:, :HD],
                    den[:],
                    start=True,
                    stop=True,
                )
                rbs = small_pool.tile([HD, QC], f32, tag="rbs", name="rbs")
                # ~51 ULP approx (plenty for softmax denominators), ~5x
                # faster than the exact DVE reciprocal
                nc.vector.reciprocal_approx_fast(out=rbs[:], in_=rbc[:])
                nc.vector.tensor_tensor(
                    atn[b][qc][hl * HD : (hl + 1) * HD, :],
                    po[:HD, :],
                    rbs[:],
                    mybir.AluOpType.mult,
                )
            # output projection for this (b, qc)
            for qb in range(QC // P):
                for nck in range(2):
                    pp = ps_mm.tile([P, TCH], f32, tag="mm", name="psmm")
                    nc.tensor.matmul(
                        pp[:, :512],
                        atn[b][qc][:, qb * P : (qb + 1) * P],
                        wp_sb[:, nck * 512 : (nck + 1) * 512],
                        start=True,
                        stop=True,
                    )
                    ot = out_pool.tile([P, 512], f32, tag="ot", name="ot")
                    nc.vector.tensor_copy(ot[:], pp[:, :512])
                    row = b * S + qc * QC + qb * P
                    nc.sync.dma_start(
                        out_d[row : row + P, nck * 512 : (nck + 1) * 512],
                        ot[:],
                    )


def _build_program(iters=1, phases='full'):
    nc = bacc.Bacc(None, target_bir_lowering=False)

    x_d = nc.dram_tensor("x", [T, D], f32, kind="ExternalInput")
    wqkv_d = nc.dram_tensor("w_qkv", [D, FQKV], f32, kind="ExternalInput")
    bqkv_d = nc.dram_tensor("b_qkv", [FQKV], f32, kind="ExternalInput")
    wp_d = nc.dram_tensor("w_proj", [P, D], f32, kind="ExternalInput")
    out_d = nc.dram_tensor("out", [T, D], f32, kind="ExternalOutput")

    with tile.TileContext(nc) as tc:
        with (
            tc.tile_pool(name="const", bufs=1) as const,
            tc.tile_pool(name="xin", bufs=4) as xin_pool,
            tc.tile_pool(name="xt", bufs=2) as xt_pool,
            tc.tile_pool(name="qkvt", bufs=1) as qkvt_pool,
            tc.tile_pool(name="vaug", bufs=1) as vaug_pool,
            tc.tile_pool(name="pt", bufs=5) as pt_pool,
            tc.tile_pool(name="atn", bufs=1) as atn_pool,
            tc.tile_pool(name="outp", bufs=3) as out_pool,
            tc.tile_pool(name="small", bufs=3) as small_pool,
            tc.tile_pool(name="ps_mm", bufs=3, space="PSUM") as ps_mm,
            tc.tile_pool(name="ps_s", bufs=3, space="PSUM") as ps_s,
            tc.tile_pool(name="ps_o", bufs=2, space="PSUM") as ps_o,
        ):
            # ---- constants ----
            # prefetch the first token chunk before the (large) weight DMAs
            # so the transpose pipeline starts immediately
            xi_pre = []
            for j in range(TCH // P):
                xi = xin_pool.tile([P, D], f32, tag="xi", name="xi")
                nc.sync.dma_start(xi[:], x_d[j * P : (j + 1) * P, :])
                xi_pre.append(xi)
            # weights: gpsimd "casting" DMA fp32 -> f32r (bit-identical move;
            # satisfies the BIR fp32r-producer rule)
            wqkv_sb = const.tile([P, KO, FQKV], MM_DT)
            wq_stage = xt_pool.tile([P, KO, FQKV], f32, tag="xt", name="xt")
            nc.sync.dma_start(
                wq_stage[:], wqkv_d.rearrange("(ko p) f -> p ko f", p=P)
            )
            nc.vector.tensor_copy(wqkv_sb[:], wq_stage[:])
            wp_sb = const.tile([P, D], MM_DT)
            wp_stage = xin_pool.tile([P, D], f32, tag="xi", name="xi")
            nc.sync.dma_start(wp_stage[:], wp_d[:])
            nc.vector.tensor_copy(wp_sb[:], wp_stage[:])
            bqkv_sb = const.tile([P, 3], f32)
            nc.sync.dma_start(bqkv_sb[:], bqkv_d.rearrange("(c p) -> p c", p=P))
            ident_f32 = const.tile([P, P], f32)
            make_identity(nc, ident_f32[:])
            identr = const.tile([P, P], MM_DT)
            nc.vector.tensor_copy(identr[:], ident_f32[:])
            # ident2[r, c] = 1 iff r == c or r == c + 64 (c < 64): slices
            # [:64] / [64:] are 64x64 identities at partition base 0 / 64,
            # for transposing the per-head V^T chunks (lhsT and rhs of a
            # matmul must share the same base partition).
            for w in range(56):
                ps_warm = ps_s.tile([P, QC], f32, tag="s", name="pss")
                nc.tensor.matmul(
                    ps_warm[:, :P], ident_f32[:], ident_f32[:],
                    start=True, stop=True,
                )
            ident2_f32 = const.tile([P, HD], f32)
            nc.gpsimd.memset(ident2_f32[:], 0.0)
            for base in (0, -HD):
                nc.gpsimd.affine_select(
                    out=ident2_f32[:],
                    in_=ident2_f32[:],
                    compare_op=mybir.AluOpType.not_equal,
                    fill=1.0,
                    base=base,
                    pattern=[[-1, HD]],
                    channel_multiplier=1,
                )
            ident2 = const.tile([P, HD], MM_DT)
            nc.vector.tensor_copy(ident2[:], ident2_f32[:])
            ones1 = const.tile([1, P], MM_DT)
            nc.vector.memset(ones1[:].bitcast(f32), 1.0)
            # mask128[k, q] = 1.0 if k <= q else 0.0
            mask128 = const.tile([P, P], f32)
            nc.gpsimd.memset(mask128[:], 1.0)
            nc.gpsimd.affine_select(
                out=mask128[:],
                in_=mask128[:],
                compare_op=mybir.AluOpType.is_ge,
                fill=0.0,
                base=0,
                pattern=[[1, P]],
                channel_multiplier=-1,
            )

            pools = (xin_pool, xt_pool, qkvt_pool, vaug_pool, pt_pool,
                     atn_pool, out_pool, small_pool, ps_mm, ps_s, ps_o)
            consts = (wqkv_sb, wp_sb, bqkv_sb, ident_f32, identr, ident2,
                      mask128, ones1, x_d, out_d, xi_pre)
            for it in range(iters):
                _emit_body(nc, tc, pools, consts, it, phases)

    nc.compile()
    return nc


_CACHE = {}


def get_program(iters=1, phases='full'):
    key = (iters, phases)
    if key not in _CACHE:
        _CACHE[key] = _build_program(iters, phases)
    return _CACHE[key]


def make_in_maps(hidden_states, c_attn_w, c_attn_b, c_proj_w):
    x = np.ascontiguousarray(
        np.asarray(hidden_states, dtype=np.float32).reshape(T, D)
    )
    wa = np.asarray(c_attn_w, dtype=np.float32)
    ba = np.asarray(c_attn_b, dtype=np.float32)
    wp = np.asarray(c_proj_w, dtype=np.float32)
    in_maps = []
    for c in range(N_CORES):
        lo, hi = c * P, (c + 1) * P
        w_qkv = np.ascontiguousarray(
            np.concatenate(
                [wa[:, lo:hi], wa[:, D + lo : D + hi], wa[:, 2 * D + lo : 2 * D + hi]],
                axis=1,
            )
        )
        b_qkv = np.ascontiguousarray(
            np.concatenate([ba[lo:hi], ba[D + lo : D + hi], ba[2 * D + lo : 2 * D + hi]])
        )
        w_proj = np.ascontiguousarray(wp[lo:hi, :])
        in_maps.append({"x": x, "w_qkv": w_qkv, "b_qkv": b_qkv, "w_proj": w_proj})
    return in_maps


def kernel(hidden_states, c_attn_w, c_attn_b, c_proj_w, c_proj_b):
    nc = get_program()
    in_maps = make_in_maps(hidden_states, c_attn_w, c_attn_b, c_proj_w)
    res = run_bass_kernel_spmd(nc, in_maps, list(range(N_CORES)))
    # unshard: row-parallel projection partials sum + bias
    acc = res.results[0]["out"]
    for c in range(1, N_CORES):
        acc = acc + res.results[c]["out"]
    acc = acc + np.asarray(c_proj_b, dtype=np.float32)[None, :]
    return acc.reshape(B, S, D).astype(np.float32)


if __name__ == "__main__":
    rng = np.random.default_rng(0)
    hs = rng.standard_normal((B, S, D), dtype=np.float32)
    wa = rng.standard_normal((D, 3 * D), dtype=np.float32) * 0.02
    ba = rng.standard_normal((3 * D,), dtype=np.float32) * 0.02
    wp = rng.standard_normal((D, D), dtype=np.float32) * 0.02
    bp = rng.standard_normal((D,), dtype=np.float32) * 0.02
    out = kernel(hs, wa, ba, wp, bp)
    print("out", out.shape, out.dtype, float(np.abs(out).max()))



# revision 3
# speedup vs baseline: 1.2701x; 1.2701x over previous
"""GPT2-style fused attention (DecisionTransformer) on 8 Trainium2 NeuronCores.

Sharding: tensor-parallel over the 16 heads (2 heads per core, both batch
elements on every core).  v2 changes vs baseline:
  - X is transposed AND cast to bf16 on the host: kills the 256 PE
    transposes (65k cycles) + 64 scalar-engine evictions (~39us) per core
    and halves the input DMA.
  - all weights arrive bf16 from the host (no on-device casts).
  - every matmul runs in bf16 (1 cycle/row incl. <256-free-dim shapes
    where fp32r pays 4x).
  - exp processes both heads per key block in one Activation instruction
    ([128, 2, 512] PSUM tile) halving Act instruction-count overhead.
  - partial outputs written bf16 (halves output DMA); host sums in fp32.

Each core:
  - loads X^T bf16 [1024, 4096] chunk-wise,
  - computes Q/K/V (transposed feature-major layout) for its 2 heads,
  - causal attention for its 4 (batch, head) pairs: scores^T = K @ Q^T,
    exp (no max subtraction -- logits are small and bounded), ones-column
    appended to V gives the softmax denominator for free in the A@V matmul,
  - row-parallel output projection with its 128 rows of c_proj_w,
  - writes a full-shape bf16 partial output [4096, 1024].
Host sums the 8 partials in fp32 (row-parallel all-reduce) and adds c_proj_b.
"""

import sys

for _p in ("/opt/trn_rl_repo",):
    if _p not in sys.path:
        sys.path.insert(0, _p)

import numpy as np

import concourse.bass as bass
import concourse.mybir as mybir
import concourse.tile as tile
from concourse import bacc
from concourse.bass_utils import run_bass_kernel_spmd
from concourse.masks import make_identity

P = 128
B, S, D, H, HD = 2, 2048, 1024, 16, 64
T = B * S              # 4096 tokens
FQKV = 3 * P           # 384 per-core qkv features (q128 | k128 | v128)
KO = D // P            # 8 contraction chunks
TCH = 512              # token chunk for qkv phase
NTCH = T // TCH        # 8
QC = 512               # query chunk in attention
NQC = S // QC          # 4
NKB = S // P           # 16 key blocks per sequence
SCALE = 1.0 / float(HD) ** 0.5
N_CORES = 8
HPC = H // N_CORES     # 2 heads per core

f32 = mybir.dt.float32
f32r = mybir.dt.float32r
bf16 = mybir.dt.bfloat16
MM_DT = bf16


def _emit_body(nc, tc, pools, consts, it, phases='full'):
    (xt_pool, qkvt_pool, vaug_pool, pt_pool, atn_pool, out_pool,
     small_pool, ps_mm, ps_s, ps_o) = pools
    (wqkv_sb, wp_sb, bqkv_sb, ident_f32, ident2, mask128,
     ones1, x_d, out_d, xt_pre) = consts

    ktb = [qkvt_pool.tile([P, S], MM_DT, tag=f"kt{b}", name=f"kt{b}")
           for b in range(B)]
    vtb = [qkvt_pool.tile([P, S], f32r, tag=f"vt{b}", name=f"vt{b}")
           for b in range(B)]
    # Q^T per (batch, local head), zero-padded to 128 contraction rows: the
    # other head's 64 rows stay zero so a full-128-partition matmul against
    # the stacked K^T contracts exactly (sub-128 matmuls run at half rate).
    qpad = [
        [qkvt_pool.tile([P, S], MM_DT, tag=f"qp{b}{h}", name=f"qp{b}{h}")
         for h in range(HPC)]
        for b in range(B)
    ]
    if it == 0:
        for b in range(B):
            nc.vector.memset(qpad[b][0][HD:, :], 0.0)
            nc.vector.memset(qpad[b][1][:HD, :], 0.0)
    vaug = [
        vaug_pool.tile([P, NKB, P], MM_DT, tag=f"vaug{p}", name=f"vaug{p}")
        for p in range(B * HPC)
    ]
    atn = [
        [
            atn_pool.tile([P, QC], MM_DT, tag=f"atn{b}_{q}", name=f"atn{b}_{q}")
            for q in range(NQC)
        ]
        for b in range(B)
    ]

    # ---- phase 1-2 per batch: QKV projection (X^T arrives pre-transposed
    # bf16 from the host), V_aug ----
    for b in range(B):
        for i in range(S // TCH):
            gi = b * (S // TCH) + i
            if it == 0 and gi == 0:
                xt = xt_pre
            else:
                xt = xt_pool.tile([P, KO, TCH], MM_DT, tag="xt", name="xt")
                nc.sync.dma_start(
                    xt[:],
                    x_d.rearrange("(ko p) t -> p ko t", p=P)[
                        :, :, gi * TCH : (gi + 1) * TCH
                    ],
                )
            for fc in range(3):
                ps = ps_mm.tile([P, TCH], f32, tag="mm", name="psmm")
                for ko in range(KO):
                    nc.tensor.matmul(
                        ps[:],
                        wqkv_sb[:, ko, fc * P : (fc + 1) * P],
                        xt[:, ko, :],
                        start=(ko == 0),
                        stop=(ko == KO - 1),
                    )
                # evict + per-partition bias add on DVE
                cs = slice(i * TCH, (i + 1) * TCH)
                if fc == 0:
                    nc.vector.tensor_scalar(
                        qpad[b][0][:HD, cs], ps[:HD],
                        bqkv_sb[:HD, fc : fc + 1], None, mybir.AluOpType.add,
                    )
                    nc.vector.tensor_scalar(
                        qpad[b][1][HD:, cs], ps[HD:],
                        bqkv_sb[HD:, fc : fc + 1], None, mybir.AluOpType.add,
                    )
                else:
                    dst = ktb[b] if fc == 1 else vtb[b]
                    nc.vector.tensor_scalar(
                        dst[:, cs], ps[:],
                        bqkv_sb[:, fc : fc + 1], None, mybir.AluOpType.add,
                    )
        # V_aug for this batch (V back to natural layout + ones column)
        for hl in range(HPC):
            p = b * HPC + hl
            vt = vtb[b][hl * HD : (hl + 1) * HD, :]
            if it == 0:
                nc.vector.memset(vaug[p][:, :, HD : HD + 1], 1.0)
                nc.vector.memset(vaug[p][:, :, HD + 1 :], 0.0)
            for kb in range(0, NKB, 2):
                ps = ps_mm.tile([P, TCH], f32, tag="mm", name="psmm")
                for u in range(2):
                    nc.tensor.transpose(
                        ps[:, u * HD : (u + 1) * HD].bitcast(f32r),
                        vt[:, (kb + u) * P : (kb + u + 1) * P],
                        ident2[hl * HD : (hl + 1) * HD, :],
                    )
                nc.vector.tensor_copy(
                    vaug[p][:, kb : kb + 2, :HD],
                    ps[:, : 2 * HD].rearrange("p (u h) -> p u h", u=2),
                )

    if phases == 'a':
        return
    # ---- phase 3+4: attention + output projection ----
    for b in range(B):
        for qc in range(NQC):
            po = [
                ps_o.tile([P, QC], f32, tag=f"po{hl}", name=f"pso{hl}")
                for hl in range(HPC)
            ]
            nkb = (qc + 1) * (QC // P)
            for kb in range(nkb):
                j = kb - qc * (QC // P)
                lo = j * P if j > 0 else 0
                # both heads' scores into one 2-bank PSUM tile; the K block
                # (both heads stacked) is the shared stationary operand
                ps2 = ps_s.tile([P, HPC, QC], f32, tag="s", name="pss")
                for hl in range(HPC):
                    nc.tensor.matmul(
                        ps2[:, hl, lo:],
                        ktb[b][:, kb * P : (kb + 1) * P],
                        qpad[b][hl][:, qc * QC + lo : (qc + 1) * QC],
                        start=True,
                        stop=True,
                    )
                pt2 = pt_pool.tile([P, HPC, QC], MM_DT, tag="pt", name="pt")
                # one exp covers both heads (free size up to 1024)
                nc.scalar.activation(
                    pt2[:, :, lo:],
                    ps2[:, :, lo:],
                    mybir.ActivationFunctionType.Exp,
                    scale=SCALE,
                )
                if j >= 0:
                    # diagonal block: zero the strictly-upper triangle
                    for hl in range(HPC):
                        nc.vector.tensor_tensor(
                            pt2[:, hl, j * P : (j + 1) * P],
                            pt2[:, hl, j * P : (j + 1) * P],
                            mask128[:],
                            mybir.AluOpType.mult,
                        )
                for hl in range(HPC):
                    nc.tensor.matmul(
                        po[hl][:, lo:],
                        vaug[b * HPC + hl][:, kb, :],
                        pt2[:, hl, lo:],
                        start=(kb == 0),
                        stop=(kb == nkb - 1),
                    )
            for hl in range(HPC):
                # normalize: A^T = O^T_u * (1/denom), denom = po[64].
                # Broadcast denom across 64 partitions FIRST (rank-1 PE
                # matmul), then reciprocal on 64 lanes -- a [1,512]
                # single-lane reciprocal measures ~3.4us on HW.
                den = small_pool.tile([1, QC], f32r, tag="rec", name="rec")
                nc.vector.tensor_copy(den[:], po[hl][HD : HD + 1, :])
                rbc = ps_mm.tile([P, TCH], f32, tag="mm", name="psmm")[:HD, :QC]
                nc.tensor.matmul(
                    rbc[:],
                    ones1[:, :HD],
                    den[:],
                    start=True,
                    stop=True,
                )
                rbs = small_pool.tile([HD, QC], f32, tag="rbs", name="rbs")
                # ~51 ULP approx (plenty for softmax denominators), ~5x
                # faster than the exact DVE reciprocal
                nc.vector.reciprocal_approx_fast(out=rbs[:], in_=rbc[:])
                nc.vector.tensor_tensor(
                    atn[b][qc][hl * HD : (hl + 1) * HD, :],
                    po[hl][:HD, :],
                    rbs[:],
                    mybir.AluOpType.mult,
                )
            # output projection for this (b, qc)
            for qb in range(QC // P):
                for nck in range(2):
                    pp = ps_mm.tile([P, TCH], f32, tag="mm", name="psmm")
                    nc.tensor.matmul(
                        pp[:, :512],
                        atn[b][qc][:, qb * P : (qb + 1) * P],
                        wp_sb[:, nck * 512 : (nck + 1) * 512],
                        start=True,
                        stop=True,
                    )
                    ot = out_pool.tile([P, 512], MM_DT, tag="ot", name="ot")
                    nc.vector.tensor_copy(ot[:], pp[:, :512])
                    row = b * S + qc * QC + qb * P
                    nc.sync.dma_start(
                        out_d[row : row + P, nck * 512 : (nck + 1) * 512],
                        ot[:],
                    )


def _build_program(iters=1, phases='full'):
    nc = bacc.Bacc(None, target_bir_lowering=False)

    x_d = nc.dram_tensor("x", [D, T], bf16, kind="ExternalInput")
    wqkv_d = nc.dram_tensor("w_qkv", [D, FQKV], bf16, kind="ExternalInput")
    bqkv_d = nc.dram_tensor("b_qkv", [FQKV], f32, kind="ExternalInput")
    wp_d = nc.dram_tensor("w_proj", [P, D], bf16, kind="ExternalInput")
    out_d = nc.dram_tensor("out", [T, D], bf16, kind="ExternalOutput")

    with tile.TileContext(nc) as tc:
        with (
            tc.tile_pool(name="const", bufs=1) as const,
            tc.tile_pool(name="xt", bufs=3) as xt_pool,
            tc.tile_pool(name="qkvt", bufs=1) as qkvt_pool,
            tc.tile_pool(name="vaug", bufs=1) as vaug_pool,
            tc.tile_pool(name="pt", bufs=4) as pt_pool,
            tc.tile_pool(name="atn", bufs=1) as atn_pool,
            tc.tile_pool(name="outp", bufs=3) as out_pool,
            tc.tile_pool(name="small", bufs=3) as small_pool,
            tc.tile_pool(name="ps_mm", bufs=2, space="PSUM") as ps_mm,
            tc.tile_pool(name="ps_s", bufs=2, space="PSUM") as ps_s,
            tc.tile_pool(name="ps_o", bufs=1, space="PSUM") as ps_o,
        ):
            # ---- constants ----
            # prefetch the first token chunk before the weight DMAs so the
            # QKV pipeline starts immediately
            xt_pre = xt_pool.tile([P, KO, TCH], MM_DT, tag="xt", name="xt")
            nc.sync.dma_start(
                xt_pre[:],
                x_d.rearrange("(ko p) t -> p ko t", p=P)[:, :, 0:TCH],
            )
            wqkv_sb = const.tile([P, KO, FQKV], MM_DT)
            nc.sync.dma_start(
                wqkv_sb[:], wqkv_d.rearrange("(ko p) f -> p ko f", p=P)
            )
            wp_sb = const.tile([P, D], MM_DT)
            nc.sync.dma_start(wp_sb[:], wp_d[:])
            bqkv_sb = const.tile([P, 3], f32)
            nc.sync.dma_start(bqkv_sb[:], bqkv_d.rearrange("(c p) -> p c", p=P))
            ident_f32 = const.tile([P, P], f32)
            make_identity(nc, ident_f32[:])
            ident_bf = const.tile([P, P], MM_DT)
            nc.vector.tensor_copy(ident_bf[:], ident_f32[:])
            # PE warmup: keep the array busy through the initial DMAs so the
            # p-state ramps and stays ramped when real matmuls arrive
            for w in range(56):
                ps_warm = ps_s.tile([P, HPC, QC], f32, tag="s", name="pss")
                nc.tensor.matmul(
                    ps_warm[:, 0, :P], ident_bf[:], ident_bf[:],
                    start=True, stop=True,
                )
            # ident2[r, c] = 1 iff r == c or r == c + 64 (c < 64): slices
            # [:64] / [64:] are 64x64 identities at partition base 0 / 64,
            # for transposing the per-head V^T chunks (lhsT and rhs of a
            # matmul must share the same base partition).
            ident2_f32 = const.tile([P, HD], f32)
            nc.gpsimd.memset(ident2_f32[:], 0.0)
            for base in (0, -HD):
                nc.gpsimd.affine_select(
                    out=ident2_f32[:],
                    in_=ident2_f32[:],
                    compare_op=mybir.AluOpType.not_equal,
                    fill=1.0,
                    base=base,
                    pattern=[[-1, HD]],
                    channel_multiplier=1,
                )
            ident2 = const.tile([P, HD], f32r)
            nc.vector.tensor_copy(ident2[:], ident2_f32[:])
            ones1 = const.tile([1, P], f32r)
            nc.vector.memset(ones1[:].bitcast(f32), 1.0)
            # mask128[k, q] = 1.0 if k <= q else 0.0
            mask128_f32 = const.tile([P, P], f32)
            nc.gpsimd.memset(mask128_f32[:], 1.0)
            nc.gpsimd.affine_select(
                out=mask128_f32[:],
                in_=mask128_f32[:],
                compare_op=mybir.AluOpType.is_ge,
                fill=0.0,
                base=0,
                pattern=[[1, P]],
                channel_multiplier=-1,
            )
            mask128 = const.tile([P, P], MM_DT)
            nc.vector.tensor_copy(mask128[:], mask128_f32[:])

            pools = (xt_pool, qkvt_pool, vaug_pool, pt_pool,
                     atn_pool, out_pool, small_pool, ps_mm, ps_s, ps_o)
            consts = (wqkv_sb, wp_sb, bqkv_sb, ident_f32, ident2,
                      mask128, ones1, x_d, out_d, xt_pre)
            for it in range(iters):
                _emit_body(nc, tc, pools, consts, it, phases)

    nc.compile()
    return nc


_CACHE = {}


def get_program(iters=1, phases='full'):
    key = (iters, phases)
    if key not in _CACHE:
        _CACHE[key] = _build_program(iters, phases)
    return _CACHE[key]


def make_in_maps(hidden_states, c_attn_w, c_attn_b, c_proj_w):
    import ml_dtypes

    bf = ml_dtypes.bfloat16
    x = np.asarray(hidden_states, dtype=np.float32).reshape(T, D)
    xt = np.ascontiguousarray(x.T.astype(bf))      # X^T [D, T] bf16
    wa = np.asarray(c_attn_w, dtype=np.float32)
    ba = np.asarray(c_attn_b, dtype=np.float32)
    wp = np.asarray(c_proj_w, dtype=np.float32)
    in_maps = []
    for c in range(N_CORES):
        lo, hi = c * P, (c + 1) * P
        w_qkv = np.ascontiguousarray(
            np.concatenate(
                [wa[:, lo:hi], wa[:, D + lo : D + hi], wa[:, 2 * D + lo : 2 * D + hi]],
                axis=1,
            ).astype(bf)
        )
        b_qkv = np.ascontiguousarray(
            np.concatenate([ba[lo:hi], ba[D + lo : D + hi], ba[2 * D + lo : 2 * D + hi]])
        )
        w_proj = np.ascontiguousarray(wp[lo:hi, :].astype(bf))
        in_maps.append({"x": xt, "w_qkv": w_qkv, "b_qkv": b_qkv, "w_proj": w_proj})
    return in_maps


def kernel(hidden_states, c_attn_w, c_attn_b, c_proj_w, c_proj_b):
    nc = get_program()
    in_maps = make_in_maps(hidden_states, c_attn_w, c_attn_b, c_proj_w)
    res = run_bass_kernel_spmd(nc, in_maps, list(range(N_CORES)))
    # unshard: row-parallel projection partials sum (fp32) + bias
    acc = np.zeros((T, D), dtype=np.float32)
    for c in range(N_CORES):
        acc += np.asarray(res.results[c]["out"], dtype=np.float32)
    acc += np.asarray(c_proj_b, dtype=np.float32)[None, :]
    return acc.reshape(B, S, D).astype(np.float32)


if __name__ == "__main__":
    rng = np.random.default_rng(0)
    hs = rng.standard_normal((B, S, D), dtype=np.float32)
    wa = rng.standard_normal((D, 3 * D), dtype=np.float32) * 0.02
    ba = rng.standard_normal((3 * D,), dtype=np.float32) * 0.02
    wp = rng.standard_normal((D, D), dtype=np.float32) * 0.02
    bp = rng.standard_normal((D,), dtype=np.float32) * 0.02
    out = kernel(hs, wa, ba, wp, bp)
    print("out", out.shape, out.dtype, float(np.abs(out).max()))


# revision 11
# speedup vs baseline: 1.3941x; 1.0976x over previous
"""GPT2-style fused attention (DecisionTransformer) on 8 Trainium2 NeuronCores.

Sharding: tensor-parallel over the 16 heads (2 heads per core, both batch
elements on every core).  v4 changes vs v3:
  - causal diagonal mask is a multiplicative 0/1 mask applied after exp on
    the (otherwise idle) GpSimd engine (GpSimd cannot touch PSUM, so the
    additive-PSUM-seed idea is out); pt lives in SBUF so this is legal.
  - ones-BLOCK denominator: V_aug columns [64,128) are all ones, so the
    A@V matmul itself materializes the softmax denominator broadcast
    across PSUM partitions 64-127 at zero extra cycles.  This removes the
    per-chunk DVE den-copy and the PE rank-1 broadcast matmul.
  - normalize (DVE) and projection (PE) for query-chunk qc are deferred
    and emitted inside qc+1's key-block loop, so the projection matmuls
    fill PE stalls while exp catches up, instead of stalling the PE queue
    at the chunk boundary.
v3: row-tiled concurrent per-head score matmuls (64-row contraction at
    tile_position (0,0)/(64,0)), Q in natural stacked layout, A@V software-
    pipelined one key block behind exp.
v2: host-side X^T in bf16, bf16 weights/matmuls everywhere, merged
    two-head exp, bf16 partial outputs summed on host in fp32.
"""

import sys

for _p in ("/opt/trn_rl_repo",):
    if _p not in sys.path:
        sys.path.insert(0, _p)

import numpy as np

import concourse.bass as bass
import concourse.mybir as mybir
import concourse.tile as tile
from concourse import bacc
from concourse.bass_utils import run_bass_kernel_spmd
from concourse.masks import make_identity

P = 128
B, S, D, H, HD = 2, 2048, 1024, 16, 64
T = B * S              # 4096 tokens
FQKV = 3 * P           # 384 per-core qkv features (q128 | k128 | v128)
KO = D // P            # 8 contraction chunks
TCH = 512              # token chunk for qkv phase
QC = 512               # query chunk in attention
NQC = S // QC          # 4
NKB = S // P           # 16 key blocks per sequence
SCALE = 1.0 / float(HD) ** 0.5
N_CORES = 8
HPC = H // N_CORES     # 2 heads per core

f32 = mybir.dt.float32
f32r = mybir.dt.float32r
bf16 = mybir.dt.bfloat16
MM_DT = bf16
# Row-tiled per-head score matmuls (64-row contraction, concurrent PE array
# halves).  Fallback (False): zero-padded Q, full-128 contraction.
ROW_TILED = True


def _emit_body(nc, tc, pools, consts, it, phases='full'):
    (xt_pool, qkvt_pool, vaug_pool, pt_pool, atn_pool, out_pool,
     small_pool, ps_mm, ps_s, ps_o) = pools
    (wqkv_sb, wp_sb, bqkv_sb, ident2, mask128, x_d, out_d, xt_pre) = consts

    ktb = [qkvt_pool.tile([P, S], MM_DT, tag=f"kt{b}", name=f"kt{b}")
           for b in range(B)]
    vtb = [qkvt_pool.tile([P, S], f32r, tag=f"vt{b}", name=f"vt{b}")
           for b in range(B)]
    if ROW_TILED:
        # Q^T natural stacked layout [h0 64 | h1 64, S] -- same as K^T; the
        # row-tiled score matmul contracts each head's 64 rows separately.
        qtb = [qkvt_pool.tile([P, S], MM_DT, tag=f"qt{b}", name=f"qt{b}")
               for b in range(B)]
    else:
        # Q^T per (batch, local head), zero-padded to 128 contraction rows
        qpad = [
            [qkvt_pool.tile([P, S], MM_DT, tag=f"qp{b}{h}", name=f"qp{b}{h}")
             for h in range(HPC)]
            for b in range(B)
        ]
        if it == 0:
            for b in range(B):
                nc.gpsimd.memset(qpad[b][0][HD:, :], 0.0)
                nc.gpsimd.memset(qpad[b][1][:HD, :], 0.0)
    vaug = [
        vaug_pool.tile([P, NKB, P], MM_DT, tag=f"vaug{p}", name=f"vaug{p}")
        for p in range(B * HPC)
    ]
    atn = [
        [
            atn_pool.tile([P, QC], MM_DT, tag=f"atn{b}_{q}", name=f"atn{b}_{q}")
            for q in range(NQC)
        ]
        for b in range(B)
    ]

    # ---- phase 1-2 per batch: QKV projection, V_aug ----
    for b in range(B):
        for i in range(S // TCH):
            gi = b * (S // TCH) + i
            if it == 0 and gi == 0:
                xt = xt_pre
            else:
                xt = xt_pool.tile([P, KO, TCH], MM_DT, tag="xt", name="xt")
                nc.sync.dma_start(
                    xt[:],
                    x_d.rearrange("(ko p) t -> p ko t", p=P)[
                        :, :, gi * TCH : (gi + 1) * TCH
                    ],
                )
            for fc in range(3):
                ps = ps_mm.tile([P, TCH], f32, tag="mm", name="psmm")
                for ko in range(KO):
                    nc.tensor.matmul(
                        ps[:],
                        wqkv_sb[:, ko, fc * P : (fc + 1) * P],
                        xt[:, ko, :],
                        start=(ko == 0),
                        stop=(ko == KO - 1),
                    )
                # evict + per-partition bias add on DVE
                cs = slice(i * TCH, (i + 1) * TCH)
                if fc == 0 and not ROW_TILED:
                    nc.vector.tensor_scalar(
                        qpad[b][0][:HD, cs], ps[:HD],
                        bqkv_sb[:HD, fc : fc + 1], None, mybir.AluOpType.add,
                    )
                    nc.vector.tensor_scalar(
                        qpad[b][1][HD:, cs], ps[HD:],
                        bqkv_sb[HD:, fc : fc + 1], None, mybir.AluOpType.add,
                    )
                else:
                    dst = (qtb[b] if ROW_TILED else None, ktb[b], vtb[b])[fc]
                    nc.vector.tensor_scalar(
                        dst[:, cs], ps[:],
                        bqkv_sb[:, fc : fc + 1], None, mybir.AluOpType.add,
                    )
        # V_aug for this batch: columns [0,64) are all ones and columns
        # [64,128) hold V in natural layout -- the A@V matmul then
        # materializes the softmax denominator broadcast across PSUM
        # partitions 0-63 at zero extra cycles.  Ones-first, because the
        # custom-DVE reciprocal misreads PSUM at base partition 64.
        for hl in range(HPC):
            p = b * HPC + hl
            vt = vtb[b][hl * HD : (hl + 1) * HD, :]
            if it == 0:
                nc.gpsimd.memset(vaug[p][:, :, :HD], 1.0)
            for kb in range(0, NKB, 2):
                ps = ps_mm.tile([P, TCH], f32, tag="mm", name="psmm")
                for u in range(2):
                    nc.tensor.transpose(
                        ps[:, u * HD : (u + 1) * HD].bitcast(f32r),
                        vt[:, (kb + u) * P : (kb + u + 1) * P],
                        ident2[hl * HD : (hl + 1) * HD, :],
                    )
                nc.vector.tensor_copy(
                    vaug[p][:, kb : kb + 2, HD:],
                    ps[:, : 2 * HD].rearrange("p (u h) -> p u h", u=2),
                )

    if phases == 'a':
        return

    # ---- phase 3+4: attention + output projection ----
    def make_norm(b, qc, po):
        def emit_norm():
            for hl in range(HPC):
                # po[0:64] holds the denominator broadcast across 64
                # partitions (ones-block trick); ~51 ULP reciprocal is
                # plenty for softmax denominators
                rbs = small_pool.tile([HD, QC], f32, tag="rbs", name="rbs")
                nc.vector.reciprocal_approx_fast(
                    out=rbs[:], in_=po[hl][:HD, :]
                )
                nc.vector.tensor_tensor(
                    atn[b][qc][hl * HD : (hl + 1) * HD, :],
                    po[hl][HD:, :],
                    rbs[:],
                    mybir.AluOpType.mult,
                )
        return emit_norm

    def make_proj(b, qc):
        def emit_proj():
            for qb in range(QC // P):
                for nck in range(2):
                    pp = ps_mm.tile([P, TCH], f32, tag="mm", name="psmm")
                    nc.tensor.matmul(
                        pp[:, :512],
                        atn[b][qc][:, qb * P : (qb + 1) * P],
                        wp_sb[:, nck * 512 : (nck + 1) * 512],
                        start=True,
                        stop=True,
                    )
                    ot = out_pool.tile([P, 512], MM_DT, tag="ot", name="ot")
                    nc.vector.tensor_copy(ot[:], pp[:, :512])
                    row = b * S + qc * QC + qb * P
                    nc.sync.dma_start(
                        out_d[row : row + P, nck * 512 : (nck + 1) * 512],
                        ot[:],
                    )
        return emit_proj

    prev_norm = prev_proj = None
    for b in range(B):
        for qc in range(NQC):
            # normalize for the previous chunk first: its po tiles must be
            # fully read before this chunk's A@V can reuse them (bufs=1)
            if prev_norm is not None:
                prev_norm()
                prev_norm = None
            po = [
                ps_o.tile([P, QC], f32, tag=f"po{hl}", name=f"pso{hl}")
                for hl in range(HPC)
            ]
            nkb = (qc + 1) * (QC // P)

            def emit_av(kb, pt2, lo):
                for hl in range(HPC):
                    nc.tensor.matmul(
                        po[hl][:, lo:],
                        vaug[b * HPC + hl][:, kb, :],
                        pt2[:, hl, lo:],
                        start=(kb == 0),
                        stop=(kb == nkb - 1),
                    )

            pending = None      # (kb, pt2, lo): A@V trails exp by one block
            for kb in range(nkb):
                j = kb - qc * (QC // P)
                lo = j * P if j > 0 else 0
                ps2 = ps_s.tile([P, HPC, QC], f32, tag="s", name="pss")
                for hl in range(HPC):
                    if ROW_TILED:
                        # row-tiled: head hl contracts rows [64*hl, ...+64);
                        # the two heads run concurrently in PE array halves
                        nc.tensor.matmul(
                            ps2[:, hl, lo:],
                            ktb[b][hl * HD : (hl + 1) * HD,
                                   kb * P : (kb + 1) * P],
                            qtb[b][hl * HD : (hl + 1) * HD,
                                   qc * QC + lo : (qc + 1) * QC],
                            start=True,
                            stop=True,
                        )
                    else:
                        nc.tensor.matmul(
                            ps2[:, hl, lo:],
                            ktb[b][:, kb * P : (kb + 1) * P],
                            qpad[b][hl][:, qc * QC + lo : (qc + 1) * QC],
                            start=True,
                            stop=True,
                        )
                pt2 = pt_pool.tile([P, HPC, QC], MM_DT, tag="pt", name="pt")
                # one exp covers both heads (free size up to 1024)
                nc.scalar.activation(
                    pt2[:, :, lo:],
                    ps2[:, :, lo:],
                    mybir.ActivationFunctionType.Exp,
                    scale=SCALE,
                )
                if j >= 0:
                    # diagonal block: zero the strictly-upper triangle on
                    # the idle GpSimd engine (SBUF-only ops there)
                    for hl in range(HPC):
                        nc.gpsimd.tensor_tensor(
                            pt2[:, hl, j * P : (j + 1) * P],
                            pt2[:, hl, j * P : (j + 1) * P],
                            mask128[:],
                            mybir.AluOpType.mult,
                        )
                if pending is not None:
                    emit_av(*pending)
                # the previous chunk's projection slots in here: the PE
                # reaches it while exp for this chunk's early blocks runs
                if kb == 1 and prev_proj is not None:
                    prev_proj()
                    prev_proj = None
                pending = (kb, pt2, lo)
            emit_av(*pending)
            prev_norm = make_norm(b, qc, po)
            prev_proj = make_proj(b, qc)
    prev_norm()
    prev_proj()


def _build_program(iters=1, phases='full'):
    nc = bacc.Bacc(None, target_bir_lowering=False)

    x_d = nc.dram_tensor("x", [D, T], bf16, kind="ExternalInput")
    wqkv_d = nc.dram_tensor("w_qkv", [D, FQKV], bf16, kind="ExternalInput")
    bqkv_d = nc.dram_tensor("b_qkv", [FQKV], f32, kind="ExternalInput")
    wp_d = nc.dram_tensor("w_proj", [P, D], bf16, kind="ExternalInput")
    out_d = nc.dram_tensor("out", [T, D], bf16, kind="ExternalOutput")

    with tile.TileContext(nc) as tc:
        with (
            tc.tile_pool(name="const", bufs=1) as const,
            tc.tile_pool(name="xt", bufs=3) as xt_pool,
            tc.tile_pool(name="qkvt", bufs=1) as qkvt_pool,
            tc.tile_pool(name="vaug", bufs=1) as vaug_pool,
            tc.tile_pool(name="pt", bufs=4) as pt_pool,
            tc.tile_pool(name="atn", bufs=1) as atn_pool,
            tc.tile_pool(name="outp", bufs=3) as out_pool,
            tc.tile_pool(name="small", bufs=3) as small_pool,
            tc.tile_pool(name="ps_mm", bufs=2, space="PSUM") as ps_mm,
            tc.tile_pool(name="ps_s", bufs=2, space="PSUM") as ps_s,
            tc.tile_pool(name="ps_o", bufs=1, space="PSUM") as ps_o,
        ):
            # ---- constants ----
            # prefetch the first token chunk before the weight DMAs so the
            # QKV pipeline starts immediately
            xt_pre = xt_pool.tile([P, KO, TCH], MM_DT, tag="xt", name="xt")
            nc.sync.dma_start(
                xt_pre[:],
                x_d.rearrange("(ko p) t -> p ko t", p=P)[:, :, 0:TCH],
            )
            wqkv_sb = const.tile([P, KO, FQKV], MM_DT)
            nc.sync.dma_start(
                wqkv_sb[:], wqkv_d.rearrange("(ko p) f -> p ko f", p=P)
            )
            wp_sb = const.tile([P, D], MM_DT)
            nc.sync.dma_start(wp_sb[:], wp_d[:])
            bqkv_sb = const.tile([P, 3], f32)
            nc.sync.dma_start(bqkv_sb[:], bqkv_d.rearrange("(c p) -> p c", p=P))
            ident_f32 = const.tile([P, P], f32)
            make_identity(nc, ident_f32[:])
            ident_bf = const.tile([P, P], MM_DT)
            nc.vector.tensor_copy(ident_bf[:], ident_f32[:])
            # PE warmup: keep the array busy through the initial DMAs so the
            # p-state ramps and stays ramped when real matmuls arrive
            for w in range(48):
                ps_warm = ps_s.tile([P, HPC, QC], f32, tag="s", name="pss")
                nc.tensor.matmul(
                    ps_warm[:, 0, :P], ident_bf[:], ident_bf[:],
                    start=True, stop=True,
                )
            # ident2[r, c] = 1 iff r == c or r == c + 64 (c < 64): slices
            # [:64] / [64:] are 64x64 identities at partition base 0 / 64,
            # for transposing the per-head V^T chunks (lhsT and rhs of a
            # matmul must share the same base partition).
            ident2_f32 = const.tile([P, HD], f32)
            nc.gpsimd.memset(ident2_f32[:], 0.0)
            for base in (0, -HD):
                nc.gpsimd.affine_select(
                    out=ident2_f32[:],
                    in_=ident2_f32[:],
                    compare_op=mybir.AluOpType.not_equal,
                    fill=1.0,
                    base=base,
                    pattern=[[-1, HD]],
                    channel_multiplier=1,
                )
            ident2 = const.tile([P, HD], f32r)
            nc.vector.tensor_copy(ident2[:], ident2_f32[:])
            # mask128[k, q] = 1.0 if k <= q else 0.0
            mask128_f32 = const.tile([P, P], f32)
            nc.gpsimd.memset(mask128_f32[:], 1.0)
            nc.gpsimd.affine_select(
                out=mask128_f32[:],
                in_=mask128_f32[:],
                compare_op=mybir.AluOpType.is_ge,
                fill=0.0,
                base=0,
                pattern=[[1, P]],
                channel_multiplier=-1,
            )
            mask128 = const.tile([P, P], MM_DT)
            nc.vector.tensor_copy(mask128[:], mask128_f32[:])

            pools = (xt_pool, qkvt_pool, vaug_pool, pt_pool,
                     atn_pool, out_pool, small_pool, ps_mm, ps_s, ps_o)
            consts = (wqkv_sb, wp_sb, bqkv_sb, ident2, mask128,
                      x_d, out_d, xt_pre)
            for it in range(iters):
                _emit_body(nc, tc, pools, consts, it, phases)

    nc.compile()
    return nc


_CACHE = {}


def get_program(iters=1, phases='full'):
    key = (iters, phases)
    if key not in _CACHE:
        _CACHE[key] = _build_program(iters, phases)
    return _CACHE[key]


def make_in_maps(hidden_states, c_attn_w, c_attn_b, c_proj_w):
    import ml_dtypes

    bf = ml_dtypes.bfloat16
    x = np.asarray(hidden_states, dtype=np.float32).reshape(T, D)
    xt = np.ascontiguousarray(x.T.astype(bf))      # X^T [D, T] bf16
    wa = np.asarray(c_attn_w, dtype=np.float32)
    ba = np.asarray(c_attn_b, dtype=np.float32)
    wp = np.asarray(c_proj_w, dtype=np.float32)
    in_maps = []
    for c in range(N_CORES):
        lo, hi = c * P, (c + 1) * P
        w_qkv = np.ascontiguousarray(
            np.concatenate(
                [wa[:, lo:hi], wa[:, D + lo : D + hi], wa[:, 2 * D + lo : 2 * D + hi]],
                axis=1,
            ).astype(bf)
        )
        b_qkv = np.ascontiguousarray(
            np.concatenate([ba[lo:hi], ba[D + lo : D + hi], ba[2 * D + lo : 2 * D + hi]])
        )
        w_proj = np.ascontiguousarray(wp[lo:hi, :].astype(bf))
        in_maps.append({"x": xt, "w_qkv": w_qkv, "b_qkv": b_qkv, "w_proj": w_proj})
    return in_maps


def kernel(hidden_states, c_attn_w, c_attn_b, c_proj_w, c_proj_b):
    nc = get_program()
    in_maps = make_in_maps(hidden_states, c_attn_w, c_attn_b, c_proj_w)
    res = run_bass_kernel_spmd(nc, in_maps, list(range(N_CORES)))
    # unshard: row-parallel projection partials sum (fp32) + bias
    acc = np.zeros((T, D), dtype=np.float32)
    for c in range(N_CORES):
        acc += np.asarray(res.results[c]["out"], dtype=np.float32)
    acc += np.asarray(c_proj_b, dtype=np.float32)[None, :]
    return acc.reshape(B, S, D).astype(np.float32)


if __name__ == "__main__":
    rng = np.random.default_rng(0)
    hs = rng.standard_normal((B, S, D), dtype=np.float32)
    wa = rng.standard_normal((D, 3 * D), dtype=np.float32) * 0.02
    ba = rng.standard_normal((3 * D,), dtype=np.float32) * 0.02
    wp = rng.standard_normal((D, D), dtype=np.float32) * 0.02
    bp = rng.standard_normal((D,), dtype=np.float32) * 0.02
    out = kernel(hs, wa, ba, wp, bp)
    print("out", out.shape, out.dtype, float(np.abs(out).max()))
